# revision 6
# baseline (speedup 1.0000x reference)
"""Trainium2 Bass kernel for nn_CircuitGnn (5-layer GATv2 + graph-LN + softmax aggregation).

Sharding: nodes partitioned contiguously across 8 cores (2500/core, padded to
2560).  Edges follow their destination node.  Per layer: each core computes
xl/xr projections for its own nodes (activations-stationary matmuls on
transposed h), AllGathers the xl "message table" (augmented with the
att-linear column per head), then runs the edge phase: indirect-DMA gather of
xl[src] rows, destination-broadcast + score + softmax(no-max) + scatter all
via PE selection-matrix matmuls.  Final graph LayerNorm + SoftmaxAggregation
are computed from per-core partial segment sums combined with two small
AllReduces; every core produces the identical [40, 64] output.
"""
import os
import sys

sys.path.insert(0, "/opt/trn_rl_repo")

import numpy as np

import concourse.bass as bass
import concourse.bacc as bacc
import concourse.tile as tile
import concourse.bass_utils as bass_utils
from concourse import mybir
from concourse.alu_op_type import AluOpType

F32 = mybir.dt.float32
I32 = mybir.dt.int32
AX = mybir.AxisListType.X

NC = 8            # cores
N = 20000         # nodes
E = 160000        # edges (before self loops)
B = 40            # graphs
HEADS = 4
C = 64            # out channels per head (also 2*feat_dim and final channel count)
NPC = N // NC     # 2500 nodes per core
P = 128
NT = (NPC + P - 1) // P   # 20 dst tiles per core
NPAD = NT * P             # 2560 padded rows per core
D_HID = HEADS * C         # 256
PAD_DST = 999.0           # sentinel: padded edge matches no dst row

_GSUB = 4                 # subtiles per group (512 edges)


def _ap(a, dims):
    """Build an AP with explicit extra free dims [step, count] appended."""
    return bass.AP(a.tensor, a.offset, list(a.ap[:1]) + [list(d) for d in dims])


# ---------------------------------------------------------------------------
# Host-side preprocessing
# ---------------------------------------------------------------------------

def _prep_edges(edge_index):
    src = np.concatenate([np.asarray(edge_index[0]), np.arange(N)]).astype(np.int64)
    dst = np.concatenate([np.asarray(edge_index[1]), np.arange(N)]).astype(np.int64)

    core = dst // NPC
    tloc = (dst % NPC) // P       # dst tile within core
    rloc = (dst % NPC) % P        # row within tile

    # bucket edges per (core, tile)
    counts = np.zeros((NC, NT), dtype=np.int64)
    np.add.at(counts, (core, tloc), 1)
    # shared subtile count per tile index (uniform program across cores)
    s_t = np.maximum(1, (counts.max(axis=0) + P - 1) // P)  # [NT]
    s_t = s_t.astype(np.int64)

    order = np.lexsort((dst,))  # sort edges by dst (stable)
    src_s, dst_s = src[order], dst[order]
    core_s = core[order]

    src_rows = np.zeros((NC, int(s_t.sum()) * P), dtype=np.int32)
    dst_locs = np.full((NC, int(s_t.sum()) * P), PAD_DST, dtype=np.float32)

    # tile slot offsets in the flat per-core edge array
    tile_off = np.concatenate([[0], np.cumsum(s_t) * P]).astype(np.int64)

    src_pad_global = (src // NPC) * NPAD + (src % NPC)
    for k in range(NC):
        m = core_s == k
        sk, dk = src_s[m], dst_s[m]
        spk = ((sk // NPC) * NPAD + (sk % NPC)).astype(np.int32)
        tk = ((dk % NPC) // P).astype(np.int64)
        rk = ((dk % NPC) % P).astype(np.float32)
        # edges are dst-sorted so tiles are contiguous; place each tile's run
        pos = 0
        for t in range(NT):
            cnt = int((tk == t).sum())
            base = tile_off[t]
            src_rows[k, base:base + cnt] = spk[pos:pos + cnt]
            dst_locs[k, base:base + cnt] = rk[pos:pos + cnt]
            pos += cnt
    s_tot = int(s_t.sum())

    # group structure: per tile, groups of up to _GSUB subtiles
    groups = []  # list of (tile, subtile_start_col, gs)
    for t in range(NT):
        s = 0
        while s < s_t[t]:
            gs = min(_GSUB, int(s_t[t]) - s)
            groups.append((t, int(tile_off[t] // P + s), gs))
            s += gs
    ngrp = len(groups)

    # dst-row arrays for DMA broadcast, one row of 512 per group
    dstrow = np.full((NC, ngrp, _GSUB * P), PAD_DST, dtype=np.float32)
    for gi, (t, col, gs) in enumerate(groups):
        sl = dst_locs[:, col * P:(col + gs) * P]
        dstrow[:, gi, :gs * P] = sl

    # resident layouts [P, s_tot]: element [p, col] = edge slot col*P + p
    src_cols = np.ascontiguousarray(
        src_rows.reshape(NC, s_tot, P).transpose(0, 2, 1))
    dst_cols = np.ascontiguousarray(
        dst_locs.reshape(NC, s_tot, P).transpose(0, 2, 1))
    return src_cols, dst_cols, dstrow, s_t, groups


def _aug_w(Wl, bl, att, heads):
    """Append per-head att-linear columns: al = xl @ att_h per head."""
    din, oc = Wl.shape
    w = np.zeros((din, oc + heads), dtype=np.float32)
    b = np.zeros((oc + heads,), dtype=np.float32)
    w[:, :oc] = Wl
    b[:oc] = bl
    for h in range(heads):
        w[:, oc + h] = Wl[:, h * C:(h + 1) * C] @ att[h]
        b[oc + h] = bl[h * C:(h + 1) * C] @ att[h]
    return w, b


def _prep_params(params):
    out = {}
    for li, name in enumerate(["l0", "l1", "l2", "l3", "l4"]):
        p = params[name]
        heads = 4 if name != "l4" else 1
        Wl = np.asarray(p["Wl"], np.float32)
        Wr = np.asarray(p["Wr"], np.float32)
        att = np.asarray(p["att"], np.float32)
        wl, bl = _aug_w(Wl, np.asarray(p["bl"], np.float32), att, heads)
        wr, br = _aug_w(Wr, np.asarray(p["br"], np.float32), att, heads)
        oc = heads * C
        wtot = oc + heads
        att08 = np.tile(0.8 * att.reshape(1, oc), (P, 1)).astype(np.float32)
        gbias = np.tile(np.asarray(p["bias"], np.float32).reshape(1, oc), (P, 1))
        # biases replicated across partitions for free-dim adds
        blb = np.tile(bl.reshape(1, wtot), (P, 1)).astype(np.float32)
        brb = np.tile(br.reshape(1, wtot), (P, 1)).astype(np.float32)
        out[name] = dict(wl=wl, wr=wr, blb=blb, brb=brb, att08=att08,
                         gbias=gbias, heads=heads, oc=oc, w=wtot)
    return out


# ---------------------------------------------------------------------------
# Device program
# ---------------------------------------------------------------------------

def _build_program(s_t, groups, layer_meta):
    nc = bacc.Bacc("TRN2", target_bir_lowering=False, num_devices=NC)
    s_tot = int(s_t.sum())
    ngrp = len(groups)

    # ---- I/O declarations
    xT_in = nc.dram_tensor("xT_in", [C, NPAD], F32, kind="ExternalInput")
    xrm_in = nc.dram_tensor("xrm_in", [NPAD, C], F32, kind="ExternalInput")
    srcids_in = nc.dram_tensor("srcids_in", [P, s_tot], I32, kind="ExternalInput")
    dstloc_in = nc.dram_tensor("dstloc_in", [P, s_tot], F32, kind="ExternalInput")
    dstrow_in = nc.dram_tensor("dstrow_in", [ngrp, _GSUB * P], F32, kind="ExternalInput")
    iota_row_in = nc.dram_tensor("iota_row_in", [P, P], F32, kind="ExternalInput")
    iota_col_in = nc.dram_tensor("iota_col_in", [P, 1], F32, kind="ExternalInput")
    ident_in = nc.dram_tensor("ident_in", [P, P], F32, kind="ExternalInput")
    grm_in = nc.dram_tensor("grm_in", [NPAD, B], F32, kind="ExternalInput")
    gcm_in = nc.dram_tensor("gcm_in", [B, NPAD], F32, kind="ExternalInput")
    rcnt_in = nc.dram_tensor("rcnt_in", [B, 1], F32, kind="ExternalInput")
    tsc_in = nc.dram_tensor("tsc_in", [P, 1], F32, kind="ExternalInput")
    lnw_in = nc.dram_tensor("lnw_in", [P, C], F32, kind="ExternalInput")
    lnb_in = nc.dram_tensor("lnb_in", [P, C], F32, kind="ExternalInput")
    win = {}
    for name, m in layer_meta.items():
        for wn in ["wl", "wr"]:
            win[name, wn] = nc.dram_tensor(
                f"{name}_{wn}", list(m[wn].shape), F32, kind="ExternalInput")
        for wn in ["blb", "brb", "att08", "gbias"]:
            win[name, wn] = nc.dram_tensor(
                f"{name}_{wn}", list(m[wn].shape), F32, kind="ExternalInput")
    out_fin = nc.dram_tensor("out_fin", [B, C], F32, kind="ExternalOutput")

    WMAX = D_HID + HEADS  # 260

    with tile.TileContext(nc) as tc:
        with tc.tile_pool(name="res", bufs=1) as res, \
             tc.tile_pool(name="work", bufs=2) as wk, \
             tc.tile_pool(name="gat", bufs=2) as gat, \
             tc.tile_pool(name="ps", bufs=1, space="PSUM") as ps, \
             tc.tile_pool(name="dram", bufs=1, space="DRAM") as dram:

            # ---- residents
            ids_src = res.tile([P, s_tot], I32, name="ids_src")
            ids_dst = res.tile([P, s_tot], F32, name="ids_dst")
            nc.sync.dma_start(out=ids_src[:], in_=srcids_in[:])
            nc.sync.dma_start(out=ids_dst[:], in_=dstloc_in[:])
            iota_row = res.tile([P, P], F32, name="iota_row")
            iota_col = res.tile([P, 1], F32, name="iota_col")
            ident = res.tile([P, P], F32, name="ident")
            nc.sync.dma_start(out=iota_row[:], in_=iota_row_in[:])
            nc.sync.dma_start(out=iota_col[:], in_=iota_col_in[:])
            nc.sync.dma_start(out=ident[:], in_=ident_in[:])
            xT = res.tile([C, NPAD], F32, name="xT")
            nc.sync.dma_start(out=xT[:], in_=xT_in[:])
            xrm = res.tile([P, NT * C], F32, name="xrm")
            for t in range(NT):
                nc.sync.dma_start(out=xrm[:, t * C:(t + 1) * C],
                                  in_=xrm_in[t * P:(t + 1) * P, :])
            hT_a = res.tile([P, 2 * NPAD], F32, name="hT_a")
            hT_b = res.tile([P, 2 * NPAD], F32, name="hT_b")
            xr_sb = res.tile([P, NT * WMAX], F32, name="xr_sb")
            hres = res.tile([P, NT * 2 * C], F32, name="hres")  # [h | h^2] per tile
            grm = res.tile([P, NT * B], F32, name="grm")
            for t in range(NT):
                nc.sync.dma_start(out=grm[:, t * B:(t + 1) * B],
                                  in_=grm_in[t * P:(t + 1) * P, :])
            gcm = res.tile([B, NPAD], F32, name="gcm")
            nc.sync.dma_start(out=gcm[:], in_=gcm_in[:])
            rcnt = res.tile([B, 1], F32, name="rcnt")
            nc.sync.dma_start(out=rcnt[:], in_=rcnt_in[:])
            tsc = res.tile([P, 1], F32, name="tsc")
            nc.sync.dma_start(out=tsc[:], in_=tsc_in[:])
            lnw = res.tile([P, C], F32, name="lnw")
            lnb = res.tile([P, C], F32, name="lnb")
            nc.sync.dma_start(out=lnw[:], in_=lnw_in[:])
            nc.sync.dma_start(out=lnb[:], in_=lnb_in[:])

            # ---- DRAM internals for collectives
            xl_own = dram.tile([NPAD, WMAX], F32, name="xl_own")
            xl_fulls = [
                dram.tile([NC * NPAD, WMAX], F32, addr_space="Shared",
                          name=f"xl_full{i}") for i in range(4)]
            xl_own4 = dram.tile([NPAD, C + 1], F32, name="xl_own4")
            xl_full4 = dram.tile([NC * NPAD, C + 1], F32, addr_space="Shared",
                                 name="xl_full4")
            st_in = dram.tile([B, P], F32, name="st_in")
            st_out = dram.tile([B, P], F32, addr_space="Shared", name="st_out")
            ws_in = dram.tile([B, P], F32, name="ws_in")
            ws_out = dram.tile([B, P], F32, addr_space="Shared", name="ws_out")

            def layer(name, lhsT_of, hT_next, li):
                m = layer_meta[name]
                heads, oc, w = m["heads"], m["oc"], m["w"]
                din = m["wl"].shape[0]
                nk = (din + P - 1) // P
                xl_o = xl_own if heads == 4 else xl_own4
                xl_f = xl_fulls[li] if heads == 4 else xl_full4

                # -- load layer weights/consts
                wl_sb = wk.tile([P, nk * w], F32, tag="wl_sb")
                wr_sb = wk.tile([P, nk * w], F32, tag="wr_sb")
                for kt in range(nk):
                    kk = min(P, din - kt * P)
                    nc.sync.dma_start(out=wl_sb[:kk, kt * w:kt * w + w],
                                      in_=win[name, "wl"][kt * P:kt * P + kk, :])
                    nc.sync.dma_start(out=wr_sb[:kk, kt * w:kt * w + w],
                                      in_=win[name, "wr"][kt * P:kt * P + kk, :])
                blb = wk.tile([P, w], F32, tag="blb")
                brb = wk.tile([P, w], F32, tag="brb")
                att08 = wk.tile([P, oc], F32, tag="att08")
                gbias = wk.tile([P, oc], F32, tag="gbias")
                nc.sync.dma_start(out=blb[:], in_=win[name, "blb"][:])
                nc.sync.dma_start(out=brb[:], in_=win[name, "brb"][:])
                nc.sync.dma_start(out=att08[:], in_=win[name, "att08"][:])
                nc.sync.dma_start(out=gbias[:], in_=win[name, "gbias"][:])

                # -- phase M: xl/xr projections for own nodes
                for t in range(NT):
                    pXR = ps.tile([P, 2048], F32, tag="zps", bufs=1)
                    for kt in range(nk):
                        kk = min(P, din - kt * P)
                        lhsT = lhsT_of(kt, t, kk)
                        nc.tensor.matmul(out=pXR[:, 0:w], lhsT=lhsT,
                                         rhs=wl_sb[:kk, kt * w:kt * w + w],
                                         start=(kt == 0), stop=(kt == nk - 1))
                        nc.tensor.matmul(out=pXR[:, 512:512 + w], lhsT=lhsT,
                                         rhs=wr_sb[:kk, kt * w:kt * w + w],
                                         start=(kt == 0), stop=(kt == nk - 1))
                    xl_t = wk.tile([P, w], F32, tag="xl_t", bufs=3)
                    nc.vector.tensor_tensor(out=xl_t[:], in0=pXR[:, 0:w],
                                            in1=blb[:], op=AluOpType.add)
                    nc.vector.tensor_tensor(out=xr_sb[:, t * WMAX:t * WMAX + w],
                                            in0=pXR[:, 512:512 + w],
                                            in1=brb[:], op=AluOpType.add)
                    nc.sync.dma_start(out=xl_o[t * P:(t + 1) * P, :], in_=xl_t[:])

                # -- AllGather the xl table
                nc.gpsimd.collective_compute(
                    "AllGather", AluOpType.bypass,
                    replica_groups=[list(range(NC))],
                    ins=[xl_o[:]], outs=[xl_f[:]])

                # -- phase E: edge aggregation per dst tile
                gi = 0
                for t in range(NT):
                    acc = ps.tile([P, 512], F32, tag="acc", bufs=2)
                    xr_t = xr_sb[:, t * WMAX:t * WMAX + w]
                    nsub = int(s_t[t])
                    ngr = (nsub + _GSUB - 1) // _GSUB
                    sdone = 0
                    for g in range(ngr):
                        t_, col, gs = groups[gi]
                        assert t_ == t
                        L = gs * P
                        dstbc = gat.tile([P, _GSUB * P], F32, tag="dstbc")
                        nc.sync.dma_start(
                            out=dstbc[:, :L],
                            in_=dstrow_in[gi:gi + 1, 0:L].to_broadcast([P, L]))
                        s01de = gat.tile([P, _GSUB * P], F32, tag="s01de")
                        nc.vector.tensor_tensor(
                            out=s01de[:, :L],
                            in0=iota_col[:].to_broadcast([P, L]),
                            in1=dstbc[:, :L], op=AluOpType.is_equal)
                        s01e = gat.tile([P, _GSUB * P], F32, tag="s01e")
                        nc.vector.tensor_tensor(
                            out=_ap(s01e[:], [[P, gs], [1, P]]),
                            in0=_ap(ids_dst[:, col:col + gs], [[1, gs], [0, P]]),
                            in1=_ap(iota_row[:], [[0, gs], [1, P]]),
                            op=AluOpType.is_equal)
                        # gather xl rows for the group's edges
                        g_t = gat.tile([P, _GSUB * WMAX], F32, tag="g_t")
                        for s in range(gs):
                            nc.gpsimd.indirect_dma_start(
                                out=g_t[:, s * w:s * w + w], out_offset=None,
                                in_=xl_f[:],
                                in_offset=bass.IndirectOffsetOnAxis(
                                    ap=ids_src[:, col + s:col + s + 1], axis=0))
                        # z = xr[dst] + xl[src] in PSUM (4 banks)
                        zps = ps.tile([P, 2048], F32, tag="zps", bufs=1)
                        for s in range(gs):
                            nc.tensor.matmul(
                                out=zps[:, 512 * s:512 * s + w],
                                lhsT=s01de[:, s * P:(s + 1) * P], rhs=xr_t,
                                start=True, stop=False)
                        for s in range(gs):
                            nc.tensor.matmul(
                                out=zps[:, 512 * s:512 * s + w],
                                lhsT=ident[:], rhs=g_t[:, s * w:s * w + w],
                                start=False, stop=True)
                        # relu(z) on ACT
                        rz = gat.tile([P, _GSUB * D_HID], F32, tag="rz")
                        nc.scalar.activation(
                            out=_ap(rz[:], [[oc, gs], [1, oc]]),
                            in_=_ap(zps[:], [[512, gs], [1, oc]]),
                            func=mybir.ActivationFunctionType.Relu)
                        # score = 0.2*lin + sum(0.8*att * relu(z))
                        tp = gat.tile([P, _GSUB * D_HID], F32, tag="tp")
                        nc.vector.tensor_tensor(
                            out=_ap(tp[:], [[oc, gs], [1, oc]]),
                            in0=_ap(rz[:], [[oc, gs], [1, oc]]),
                            in1=_ap(att08[:], [[0, gs], [1, oc]]),
                            op=AluOpType.mult)
                        sc = gat.tile([P, _GSUB * HEADS], F32, tag="sc")
                        nc.vector.tensor_reduce(
                            out=sc[:, :gs * heads],
                            in_=_ap(tp[:], [[C, gs * heads], [1, C]]),
                            axis=AX, op=AluOpType.add)
                        sc2 = gat.tile([P, _GSUB * HEADS], F32, tag="sc2")
                        nc.vector.scalar_tensor_tensor(
                            out=sc2[:, :gs * heads],
                            in0=_ap(bass.AP(zps[:].tensor, zps[:].offset + oc,
                                            [zps[:].ap[0]]),
                                    [[512, gs], [1, heads]]),
                            scalar=0.2, in1=sc[:, :gs * heads],
                            op0=AluOpType.mult, op1=AluOpType.add)
                        # msg = [xl * exp(score) | exp(score)]
                        msg = gat.tile([P, _GSUB * WMAX], F32, tag="msg")
                        nc.scalar.activation(
                            out=_ap(bass.AP(msg[:].tensor, msg[:].offset + oc,
                                            [msg[:].ap[0]]),
                                    [[w, gs], [1, heads]]),
                            in_=sc2[:, :gs * heads],
                            func=mybir.ActivationFunctionType.Exp)
                        nc.vector.tensor_tensor(
                            out=_ap(msg[:], [[w, gs], [1, oc]]),
                            in0=_ap(g_t[:], [[w, gs], [1, oc]]),
                            in1=_ap(bass.AP(msg[:].tensor, msg[:].offset + oc,
                                            [msg[:].ap[0]]),
                                    [[w, gs], [1, heads], [0, C]]),
                            op=AluOpType.mult)
                        for s in range(gs):
                            nc.tensor.matmul(
                                out=acc[:, 0:w],
                                lhsT=s01e[:, s * P:(s + 1) * P],
                                rhs=msg[:, s * w:s * w + w],
                                start=(sdone == 0 and s == 0),
                                stop=(g == ngr - 1 and s == gs - 1))
                        sdone += gs
                        gi += 1
                    # -- epilogue: h = leaky(acc/denom + bias) (+ residual on l4)
                    dn = wk.tile([P, HEADS], F32, tag="dn")
                    nc.vector.tensor_scalar_max(out=dn[:, :heads],
                                                in0=acc[:, oc:oc + heads],
                                                scalar1=1e-30)
                    rdn = wk.tile([P, HEADS], F32, tag="rdn")
                    nc.vector.reciprocal(out=rdn[:, :heads], in_=dn[:, :heads])
                    u = wk.tile([P, D_HID], F32, tag="u")
                    nc.vector.tensor_tensor(
                        out=u[:, :oc], in0=acc[:, 0:oc],
                        in1=_ap(rdn[:, :heads], [[1, heads], [0, C]]),
                        op=AluOpType.mult)
                    v = wk.tile([P, D_HID], F32, tag="v")
                    nc.vector.tensor_tensor(out=v[:, :oc], in0=u[:, :oc],
                                            in1=gbias[:], op=AluOpType.add)
                    if name != "l4":
                        hn = wk.tile([P, D_HID], F32, tag="hn")
                        nc.vector.scalar_tensor_tensor(
                            out=hn[:, :oc], in0=v[:, :oc], scalar=0.01,
                            in1=v[:, :oc], op0=AluOpType.mult, op1=AluOpType.max)
                        for j in range(oc // P):
                            tps = ps.tile([P, P], F32, tag="tps", bufs=2)
                            nc.tensor.transpose(tps[:], hn[:, j * P:(j + 1) * P],
                                                ident[:])
                            nc.vector.tensor_copy(
                                out=hT_next[:, j * NPAD + t * P:
                                            j * NPAD + (t + 1) * P],
                                in_=tps[:])
                    else:
                        hn = wk.tile([P, C], F32, tag="hn4")
                        nc.vector.scalar_tensor_tensor(
                            out=hn[:], in0=v[:, :oc], scalar=0.01,
                            in1=v[:, :oc], op0=AluOpType.mult, op1=AluOpType.max)
                        # residual + squares into the [h | h^2] resident
                        hrt = hres[:, t * 2 * C:t * 2 * C + C]
                        nc.vector.tensor_tensor(
                            out=hrt, in0=hn[:], in1=xrm[:, t * C:(t + 1) * C],
                            op=AluOpType.add)
                        nc.vector.tensor_tensor(
                            out=hres[:, t * 2 * C + C:(t + 1) * 2 * C],
                            in0=hrt, in1=hrt, op=AluOpType.mult)

            # ---- run the 5 layers
            def lhsT_x(kt, t, kk):
                return xT[0:kk, t * P:(t + 1) * P]

            def lhsT_of(hT):
                def f(kt, t, kk):
                    return hT[0:kk, kt * NPAD + t * P:kt * NPAD + (t + 1) * P]
                return f

            layer("l0", lhsT_x, hT_a, 0)
            layer("l1", lhsT_of(hT_a), hT_b, 1)
            layer("l2", lhsT_of(hT_b), hT_a, 2)
            layer("l3", lhsT_of(hT_a), hT_b, 3)
            layer("l4", lhsT_of(hT_b), None, 4)

            # ---- final: graph LayerNorm stats via partial sums + AllReduce
            pst = ps.tile([B, P], F32, tag="acc", bufs=2)
            for t in range(NT):
                nc.tensor.matmul(out=pst[:], lhsT=grm[:, t * B:(t + 1) * B],
                                 rhs=hres[:, t * 2 * C:(t + 1) * 2 * C],
                                 start=(t == 0), stop=(t == NT - 1))
            st_sb = wk.tile([B, P], F32, tag="st_sb")
            nc.vector.tensor_copy(out=st_sb[:], in_=pst[:])
            nc.sync.dma_start(out=st_in[:], in_=st_sb[:])
            nc.gpsimd.collective_compute(
                "AllReduce", AluOpType.add, replica_groups=[list(range(NC))],
                ins=[st_in[:]], outs=[st_out[:]])
            st_r = wk.tile([B, P], F32, tag="st_r")
            nc.sync.dma_start(out=st_r[:], in_=st_out[:])

            ssum = wk.tile([B, 4], F32, tag="ssum")
            nc.vector.tensor_reduce(out=ssum[:, 0:1], in_=st_r[:, 0:C],
                                    axis=AX, op=AluOpType.add)
            nc.vector.tensor_reduce(out=ssum[:, 1:2], in_=st_r[:, C:2 * C],
                                    axis=AX, op=AluOpType.add)
            mean = wk.tile([B, 1], F32, tag="mean")
            nc.vector.tensor_scalar(out=mean[:], in0=ssum[:, 0:1],
                                    scalar1=rcnt[:], scalar2=None,
                                    op0=AluOpType.mult)
            var = wk.tile([B, 1], F32, tag="var")
            nc.vector.tensor_scalar(out=var[:], in0=ssum[:, 1:2],
                                    scalar1=rcnt[:], scalar2=None,
                                    op0=AluOpType.mult)
            m2 = wk.tile([B, 1], F32, tag="m2")
            nc.vector.tensor_tensor(out=m2[:], in0=mean[:], in1=mean[:],
                                    op=AluOpType.mult)
            nc.vector.tensor_tensor(out=var[:], in0=var[:], in1=m2[:],
                                    op=AluOpType.subtract)
            nc.vector.tensor_scalar_add(out=var[:], in0=var[:], scalar1=1e-5)
            rv = wk.tile([B, 1], F32, tag="rv")
            nc.vector.reciprocal(out=rv[:], in_=var[:])
            rsq = wk.tile([B, 1], F32, tag="rsq")
            nc.scalar.activation(out=rsq[:], in_=rv[:],
                                 func=mybir.ActivationFunctionType.Sqrt)
            svec = wk.tile([B, 2], F32, tag="svec")
            nc.vector.tensor_copy(out=svec[:, 0:1], in_=rsq[:])
            nc.vector.tensor_tensor(out=svec[:, 1:2], in0=mean[:], in1=rsq[:],
                                    op=AluOpType.mult)
            nc.vector.tensor_scalar_mul(out=svec[:, 1:2], in0=svec[:, 1:2],
                                        scalar1=-1.0)

            # per-tile: hln, exp, weighted sums
            psw = ps.tile([B, P], F32, tag="acc", bufs=2)
            for t in range(NT):
                psb = ps.tile([P, 2], F32, tag="tps", bufs=2)
                nc.tensor.matmul(out=psb[:], lhsT=gcm[:, t * P:(t + 1) * P],
                                 rhs=svec[:], start=True, stop=True)
                sbv = wk.tile([P, 2], F32, tag="sbv")
                nc.vector.tensor_copy(out=sbv[:], in_=psb[:])
                h1 = wk.tile([P, C], F32, tag="h1")
                nc.scalar.activation(out=h1[:],
                                     in_=hres[:, t * 2 * C:t * 2 * C + C],
                                     func=mybir.ActivationFunctionType.Identity,
                                     bias=sbv[:, 1:2], scale=sbv[:, 0:1])
                hl = wk.tile([P, C], F32, tag="hl")
                nc.vector.tensor_tensor(out=hl[:], in0=h1[:], in1=lnw[:],
                                        op=AluOpType.mult)
                nc.vector.tensor_tensor(out=hl[:], in0=hl[:], in1=lnb[:],
                                        op=AluOpType.add)
                wst = wk.tile([P, 2 * C], F32, tag="wst")
                nc.scalar.activation(out=wst[:, 0:C], in_=hl[:],
                                     func=mybir.ActivationFunctionType.Exp,
                                     scale=tsc[:])
                nc.vector.tensor_tensor(out=wst[:, C:2 * C], in0=wst[:, 0:C],
                                        in1=hl[:], op=AluOpType.mult)
                nc.tensor.matmul(out=psw[:], lhsT=grm[:, t * B:(t + 1) * B],
                                 rhs=wst[:], start=(t == 0), stop=(t == NT - 1))
            ws_sb = wk.tile([B, P], F32, tag="ws_sb")
            nc.vector.tensor_copy(out=ws_sb[:], in_=psw[:])
            nc.sync.dma_start(out=ws_in[:], in_=ws_sb[:])
            nc.gpsimd.collective_compute(
                "AllReduce", AluOpType.add, replica_groups=[list(range(NC))],
                ins=[ws_in[:]], outs=[ws_out[:]])
            ws_r = wk.tile([B, P], F32, tag="ws_r")
            nc.sync.dma_start(out=ws_r[:], in_=ws_out[:])
            dnf = wk.tile([B, C], F32, tag="dnf")
            nc.vector.tensor_scalar_add(out=dnf[:], in0=ws_r[:, 0:C],
                                        scalar1=1e-16)
            rrf = wk.tile([B, C], F32, tag="rrf")
            nc.vector.reciprocal(out=rrf[:], in_=dnf[:])
            outv = wk.tile([B, C], F32, tag="outv")
            nc.vector.tensor_tensor(out=outv[:], in0=ws_r[:, C:2 * C],
                                    in1=rrf[:], op=AluOpType.mult)
            nc.sync.dma_start(out=out_fin[:], in_=outv[:])

    nc.compile()
    return nc


# ---------------------------------------------------------------------------
# Entry point
# ---------------------------------------------------------------------------

def kernel(x, edge_index, batch, params):
    x = np.asarray(x, np.float32)
    batch_np = np.asarray(batch).astype(np.int64)
    src_cols, dst_cols, dstrow, s_t, groups = _prep_edges(np.asarray(edge_index))
    lm = _prep_params(params)

    nc = _build_program(s_t, groups, lm)

    # graph one-hots / constants
    cnt = np.zeros(B); np.add.at(cnt, batch_np, 1.0)
    rcnt = (1.0 / np.maximum(cnt * C, 1.0)).astype(np.float32).reshape(B, 1)
    t_val = float(np.asarray(params["aggr_t"]))
    tsc = np.full((P, 1), t_val, np.float32)
    lnw = np.tile(np.asarray(params["ln_w"], np.float32).reshape(1, C), (P, 1))
    lnb = np.tile(np.asarray(params["ln_b"], np.float32).reshape(1, C), (P, 1))
    iota_row = np.tile(np.arange(P, dtype=np.float32)[None, :], (P, 1))
    iota_col = np.arange(P, dtype=np.float32).reshape(P, 1)
    ident = np.eye(P, dtype=np.float32)

    in_maps = []
    for k in range(NC):
        lo, hi = k * NPC, (k + 1) * NPC
        xs = np.zeros((NPAD, C), np.float32)
        xs[:NPC] = x[lo:hi]
        gr = np.zeros((NPAD, B), np.float32)
        gr[np.arange(NPC), batch_np[lo:hi]] = 1.0
        im = {
            "xT_in": np.ascontiguousarray(xs.T),
            "xrm_in": xs,
            "srcids_in": src_cols[k],
            "dstloc_in": dst_cols[k],
            "dstrow_in": dstrow[k],
            "iota_row_in": iota_row,
            "iota_col_in": iota_col,
            "ident_in": ident,
            "grm_in": gr,
            "gcm_in": np.ascontiguousarray(gr.T),
            "rcnt_in": rcnt,
            "tsc_in": tsc,
            "lnw_in": lnw,
            "lnb_in": lnb,
        }
        for name, m in lm.items():
            for wn in ["wl", "wr", "blb", "brb", "att08", "gbias"]:
                im[f"{name}_{wn}"] = np.ascontiguousarray(m[wn])
        in_maps.append(im)

    trace = os.environ.get("KERNEL_TRACE", "0") == "1"
    res = bass_utils.run_bass_kernel_spmd(
        nc, in_maps, core_ids=list(range(NC)), trace=trace)
    kernel.last_results = res
    return np.asarray(res.results[0]["out_fin"], np.float32)


# revision 11
# speedup vs baseline: 1.6824x; 1.6824x over previous
"""Trainium2 Bass kernel for nn_CircuitGnn (5-layer GATv2 + graph-LN + softmax aggregation).

Sharding: nodes partitioned contiguously across 8 cores (2500/core, padded to
2560).  Edges follow their destination node.  Per layer: each core computes
xl/xr projections for its own nodes (activations-stationary bf16 matmuls on
transposed h), AllGathers the bf16 xl "message table" (augmented with the
att-linear column per head), then runs the edge phase: one batched dma_gather
of xl[src] rows per dst tile, destination-broadcast + score +
softmax(no-max) + scatter all via PE selection-matrix matmuls (selection
matrices are 0/1 so bf16 is exact; PSUM accumulation is fp32).  Final graph
LayerNorm + SoftmaxAggregation are computed from per-core partial segment
sums combined with two small AllReduces; every core produces the identical
[40, 64] output.
"""
import os
import sys

sys.path.insert(0, "/opt/trn_rl_repo")

import numpy as np
import ml_dtypes

import concourse.bass as bass
import concourse.bacc as bacc
import concourse.tile as tile
import concourse.bass_utils as bass_utils
from concourse import mybir
from concourse.alu_op_type import AluOpType

F32 = mybir.dt.float32
BF16 = mybir.dt.bfloat16
I32 = mybir.dt.int32
I16 = mybir.dt.int16
AX = mybir.AxisListType.X
NPBF = ml_dtypes.bfloat16

NC = 8            # cores
N = 20000         # nodes
E = 160000        # edges (before self loops)
B = 40            # graphs
HEADS = 4
C = 64            # out channels per head (also 2*feat_dim and final channel count)
NPC = N // NC     # 2500 nodes per core
P = 128
NT = (NPC + P - 1) // P   # 20 dst tiles per core
NPAD = NT * P             # 2560 padded rows per core
D_HID = HEADS * C         # 256
PAD_DST = 999.0           # sentinel: padded edge matches no dst row
TW = 260                  # gather-table row width (l0-l3), 520B in bf16
TW4 = 65                  # gather-table row width for l4

_GSUB = 4                 # subtiles per group (512 edges)


def _ap(a, dims):
    """Build an AP with explicit extra free dims [step, count] appended."""
    return bass.AP(a.tensor, a.offset, list(a.ap[:1]) + [list(d) for d in dims])


def _ap_off(a, off, dims):
    return bass.AP(a.tensor, a.offset + off, list(a.ap[:1]) + [list(d) for d in dims])


# ---------------------------------------------------------------------------
# Host-side preprocessing
# ---------------------------------------------------------------------------

def _prep_edges(edge_index):
    src = np.concatenate([np.asarray(edge_index[0]), np.arange(N)]).astype(np.int64)
    dst = np.concatenate([np.asarray(edge_index[1]), np.arange(N)]).astype(np.int64)

    core = dst // NPC
    tloc = (dst % NPC) // P       # dst tile within core

    counts = np.zeros((NC, NT), dtype=np.int64)
    np.add.at(counts, (core, tloc), 1)
    # shared subtile count per tile index (uniform program across cores)
    s_t = np.maximum(1, (counts.max(axis=0) + P - 1) // P).astype(np.int64)

    order = np.lexsort((dst,))
    src_s, dst_s = src[order], dst[order]
    core_s = core[order]

    s_tot = int(s_t.sum())
    src_rows = np.zeros((NC, s_tot * P), dtype=np.int32)
    dst_locs = np.full((NC, s_tot * P), PAD_DST, dtype=np.float32)
    tile_off = np.concatenate([[0], np.cumsum(s_t) * P]).astype(np.int64)

    for k in range(NC):
        m = core_s == k
        sk, dk = src_s[m], dst_s[m]
        spk = ((sk // NPC) * NPAD + (sk % NPC)).astype(np.int32)
        tk = ((dk % NPC) // P).astype(np.int64)
        rk = ((dk % NPC) % P).astype(np.float32)
        pos = 0
        for t in range(NT):
            cnt = int((tk == t).sum())
            base = tile_off[t]
            src_rows[k, base:base + cnt] = spk[pos:pos + cnt]
            dst_locs[k, base:base + cnt] = rk[pos:pos + cnt]
            pos += cnt

    # group structure: per tile, groups of up to _GSUB subtiles
    groups = []  # (tile, subtile_start_col, gs)
    for t in range(NT):
        s = 0
        while s < s_t[t]:
            gs = min(_GSUB, int(s_t[t]) - s)
            groups.append((t, int(tile_off[t] // P + s), gs))
            s += gs
    ngrp = len(groups)

    # dst-row arrays for DMA broadcast, one row of 512 per group (bf16)
    dstrow = np.full((NC, ngrp, _GSUB * P), PAD_DST, dtype=np.float32)
    for gi, (t, col, gs) in enumerate(groups):
        dstrow[:, gi, :gs * P] = dst_locs[:, col * P:(col + gs) * P]

    srcw = np.ascontiguousarray(
        src_rows.reshape(NC, s_tot, P).transpose(0, 2, 1))  # [P, s_tot] i32

    dst_cols = np.ascontiguousarray(
        dst_locs.reshape(NC, s_tot, P).transpose(0, 2, 1)).astype(NPBF)
    return srcw, dst_cols, dstrow.astype(NPBF), s_t, groups


def _aug_w(Wl, bl, att, heads):
    din, oc = Wl.shape
    w = np.zeros((din, oc + heads), dtype=np.float32)
    b = np.zeros((oc + heads,), dtype=np.float32)
    w[:, :oc] = Wl
    b[:oc] = bl
    for h in range(heads):
        w[:, oc + h] = Wl[:, h * C:(h + 1) * C] @ att[h]
        b[oc + h] = bl[h * C:(h + 1) * C] @ att[h]
    return w, b


def _prep_params(params):
    out = {}
    for name in ["l0", "l1", "l2", "l3", "l4"]:
        p = params[name]
        heads = 4 if name != "l4" else 1
        Wl = np.asarray(p["Wl"], np.float32)
        Wr = np.asarray(p["Wr"], np.float32)
        att = np.asarray(p["att"], np.float32)
        wl, bl = _aug_w(Wl, np.asarray(p["bl"], np.float32), att, heads)
        wr, br = _aug_w(Wr, np.asarray(p["br"], np.float32), att, heads)
        oc = heads * C
        wtot = oc + heads
        att08 = np.tile(0.8 * att.reshape(1, oc), (P, 1)).astype(NPBF)
        gbias = np.tile(np.asarray(p["bias"], np.float32).reshape(1, oc), (P, 1))
        blb = np.tile(bl.reshape(1, wtot), (P, 1)).astype(np.float32)
        brb = np.tile(br.reshape(1, wtot), (P, 1)).astype(np.float32)
        out[name] = dict(wl=wl.astype(NPBF), wr=wr.astype(NPBF), blb=blb,
                         brb=brb, att08=att08, gbias=gbias, heads=heads,
                         oc=oc, w=wtot)
    return out


# ---------------------------------------------------------------------------
# Device program
# ---------------------------------------------------------------------------

def _build_program(s_t, groups, layer_meta):
    nc = bacc.Bacc("TRN2", target_bir_lowering=False, num_devices=NC)
    s_tot = int(s_t.sum())
    ngrp = len(groups)

    xT_in = nc.dram_tensor("xT_in", [C, NPAD], BF16, kind="ExternalInput")
    xrm_in = nc.dram_tensor("xrm_in", [NPAD, C], F32, kind="ExternalInput")
    srcw_in = nc.dram_tensor("srcw_in", [P, s_tot], I32, kind="ExternalInput")
    dstloc_in = nc.dram_tensor("dstloc_in", [P, s_tot], BF16, kind="ExternalInput")
    dstrow_in = nc.dram_tensor("dstrow_in", [ngrp, _GSUB * P], BF16,
                               kind="ExternalInput")
    iota_row_in = nc.dram_tensor("iota_row_in", [P, P], BF16, kind="ExternalInput")
    iota_col_in = nc.dram_tensor("iota_col_in", [P, 1], BF16, kind="ExternalInput")
    ident_in = nc.dram_tensor("ident_in", [P, P], BF16, kind="ExternalInput")
    grm_in = nc.dram_tensor("grm_in", [NPAD, B], F32, kind="ExternalInput")
    gcm_in = nc.dram_tensor("gcm_in", [B, NPAD], F32, kind="ExternalInput")
    rcnt_in = nc.dram_tensor("rcnt_in", [B, 1], F32, kind="ExternalInput")
    tsc_in = nc.dram_tensor("tsc_in", [P, 1], F32, kind="ExternalInput")
    lnw_in = nc.dram_tensor("lnw_in", [P, C], F32, kind="ExternalInput")
    lnb_in = nc.dram_tensor("lnb_in", [P, C], F32, kind="ExternalInput")
    win = {}
    for name, m in layer_meta.items():
        for wn in ["wl", "wr", "att08"]:
            win[name, wn] = nc.dram_tensor(
                f"{name}_{wn}", list(m[wn].shape), BF16, kind="ExternalInput")
        for wn in ["blb", "brb", "gbias"]:
            win[name, wn] = nc.dram_tensor(
                f"{name}_{wn}", list(m[wn].shape), F32, kind="ExternalInput")
    out_fin = nc.dram_tensor("out_fin", [B, C], F32, kind="ExternalOutput")

    WMAX = D_HID + HEADS  # 260
    SMAX = int(s_t.max())

    with tile.TileContext(nc) as tc:
        with tc.tile_pool(name="res", bufs=1) as res, \
             tc.tile_pool(name="work", bufs=2) as wk, \
             tc.tile_pool(name="gat", bufs=2) as gat, \
             tc.tile_pool(name="ps", bufs=1, space="PSUM") as ps, \
             tc.tile_pool(name="dram", bufs=1, space="DRAM") as dram:

            # ---- residents
            ids_src = res.tile([P, s_tot], I32, name="ids_src")
            ids_dst = res.tile([P, s_tot], BF16, name="ids_dst")
            nc.sync.dma_start(out=ids_src[:], in_=srcw_in[:])
            nc.sync.dma_start(out=ids_dst[:], in_=dstloc_in[:])
            iota_row = res.tile([P, P], BF16, name="iota_row")
            iota_col = res.tile([P, 1], BF16, name="iota_col")
            ident = res.tile([P, P], BF16, name="ident")
            nc.sync.dma_start(out=iota_row[:], in_=iota_row_in[:])
            nc.sync.dma_start(out=iota_col[:], in_=iota_col_in[:])
            nc.sync.dma_start(out=ident[:], in_=ident_in[:])
            xT = res.tile([C, NPAD], BF16, name="xT")
            nc.sync.dma_start(out=xT[:], in_=xT_in[:])
            xrm = res.tile([P, NT * C], F32, name="xrm")
            for t in range(NT):
                nc.sync.dma_start(out=xrm[:, t * C:(t + 1) * C],
                                  in_=xrm_in[t * P:(t + 1) * P, :])
            hT_a = res.tile([P, 2 * NPAD], BF16, name="hT_a")
            hT_b = res.tile([P, 2 * NPAD], BF16, name="hT_b")
            xr_sb = res.tile([P, NT * WMAX], BF16, name="xr_sb")
            hres = res.tile([P, NT * 2 * C], F32, name="hres")
            grm = res.tile([P, NT * B], F32, name="grm")
            for t in range(NT):
                nc.sync.dma_start(out=grm[:, t * B:(t + 1) * B],
                                  in_=grm_in[t * P:(t + 1) * P, :])
            gcm = res.tile([B, NPAD], F32, name="gcm")
            nc.sync.dma_start(out=gcm[:], in_=gcm_in[:])
            rcnt = res.tile([B, 1], F32, name="rcnt")
            nc.sync.dma_start(out=rcnt[:], in_=rcnt_in[:])
            tsc = res.tile([P, 1], F32, name="tsc")
            nc.sync.dma_start(out=tsc[:], in_=tsc_in[:])
            lnw = res.tile([P, C], F32, name="lnw")
            lnb = res.tile([P, C], F32, name="lnb")
            nc.sync.dma_start(out=lnw[:], in_=lnw_in[:])
            nc.sync.dma_start(out=lnb[:], in_=lnb_in[:])

            # ---- DRAM internals for collectives
            xl_own = dram.tile([NPAD, TW], BF16, name="xl_own")
            xl_fulls = [
                dram.tile([NC * NPAD, TW], BF16, addr_space="Shared",
                          name=f"xl_full{i}") for i in range(4)]
            xl_own4 = dram.tile([NPAD, TW4], BF16, name="xl_own4")
            xl_full4 = dram.tile([NC * NPAD, TW4], BF16, addr_space="Shared",
                                 name="xl_full4")
            st_in = dram.tile([B, P], F32, name="st_in")
            st_out = dram.tile([B, P], F32, addr_space="Shared", name="st_out")
            ws_in = dram.tile([B, P], F32, name="ws_in")
            ws_out = dram.tile([B, P], F32, addr_space="Shared", name="ws_out")

            def layer(name, lhsT_of, hT_next, li):
                m = layer_meta[name]
                heads, oc, w = m["heads"], m["oc"], m["w"]
                din = m["wl"].shape[0]
                nk = (din + P - 1) // P
                tw = TW if heads == 4 else TW4
                xl_o = xl_own if heads == 4 else xl_own4
                xl_f = xl_fulls[li] if heads == 4 else xl_full4

                wl_sb = wk.tile([P, nk * w], BF16, tag="wl_sb")
                wr_sb = wk.tile([P, nk * w], BF16, tag="wr_sb")
                for kt in range(nk):
                    kk = min(P, din - kt * P)
                    nc.sync.dma_start(out=wl_sb[:kk, kt * w:kt * w + w],
                                      in_=win[name, "wl"][kt * P:kt * P + kk, :])
                    nc.sync.dma_start(out=wr_sb[:kk, kt * w:kt * w + w],
                                      in_=win[name, "wr"][kt * P:kt * P + kk, :])
                blb = wk.tile([P, w], F32, tag="blb")
                brb = wk.tile([P, w], F32, tag="brb")
                att08 = wk.tile([P, oc], BF16, tag="att08")
                gbias = wk.tile([P, oc], F32, tag="gbias")
                nc.sync.dma_start(out=blb[:], in_=win[name, "blb"][:])
                nc.sync.dma_start(out=brb[:], in_=win[name, "brb"][:])
                nc.sync.dma_start(out=att08[:], in_=win[name, "att08"][:])
                nc.sync.dma_start(out=gbias[:], in_=win[name, "gbias"][:])

                # -- phase M: xl/xr projections for own nodes
                for t in range(NT):
                    pXR = ps.tile([P, 2048], F32, tag="zps", bufs=1)
                    for kt in range(nk):
                        kk = min(P, din - kt * P)
                        lhsT = lhsT_of(kt, t, kk)
                        nc.tensor.matmul(out=pXR[:, 0:w], lhsT=lhsT,
                                         rhs=wl_sb[:kk, kt * w:kt * w + w],
                                         start=(kt == 0), stop=(kt == nk - 1))
                        nc.tensor.matmul(out=pXR[:, 1024:1024 + w], lhsT=lhsT,
                                         rhs=wr_sb[:kk, kt * w:kt * w + w],
                                         start=(kt == 0), stop=(kt == nk - 1))
                    xl_t = wk.tile([P, w], BF16, tag="xl_t", bufs=3)
                    nc.vector.tensor_tensor(out=xl_t[:], in0=pXR[:, 0:w],
                                            in1=blb[:], op=AluOpType.add)
                    nc.vector.tensor_tensor(out=xr_sb[:, t * WMAX:t * WMAX + w],
                                            in0=pXR[:, 1024:1024 + w],
                                            in1=brb[:], op=AluOpType.add)
                    nc.sync.dma_start(out=xl_o[t * P:(t + 1) * P, 0:w], in_=xl_t[:])

                nc.gpsimd.collective_compute(
                    "AllGather", AluOpType.bypass,
                    replica_groups=[list(range(NC))],
                    ins=[xl_o[:]], outs=[xl_f[:]])

                # -- phase E
                gi = 0
                coff = 0  # subtile column offset
                for t in range(NT):
                    nsub = int(s_t[t])
                    acc = ps.tile([P, 512], F32, tag="acc", bufs=2)
                    xr_t = xr_sb[:, t * WMAX:t * WMAX + w]
                    # gather xl rows for the whole dst tile (per subtile)
                    g_t = gat.tile([P, SMAX * TW], BF16, tag="g_t")
                    for s in range(nsub):
                        nc.gpsimd.indirect_dma_start(
                            out=g_t[:, s * tw:s * tw + tw], out_offset=None,
                            in_=xl_f[:],
                            in_offset=bass.IndirectOffsetOnAxis(
                                ap=ids_src[:, coff + s:coff + s + 1], axis=0))
                    ngr = (nsub + _GSUB - 1) // _GSUB
                    sdone = 0
                    for g in range(ngr):
                        t_, col, gs = groups[gi]
                        assert t_ == t and col == coff + sdone
                        L = gs * P
                        dstbc = gat.tile([P, _GSUB * P], BF16, tag="dstbc")
                        nc.sync.dma_start(
                            out=dstbc[:, :L],
                            in_=dstrow_in[gi:gi + 1, 0:L].to_broadcast([P, L]))
                        s01de = gat.tile([P, _GSUB * P], BF16, tag="s01de")
                        nc.vector.tensor_tensor(
                            out=s01de[:, :L],
                            in0=iota_col[:].to_broadcast([P, L]),
                            in1=dstbc[:, :L], op=AluOpType.is_equal)
                        s01e = gat.tile([P, _GSUB * P], BF16, tag="s01e")
                        nc.vector.tensor_tensor(
                            out=_ap(s01e[:], [[P, gs], [1, P]]),
                            in0=_ap(ids_dst[:, col:col + gs], [[1, gs], [0, P]]),
                            in1=_ap(iota_row[:], [[0, gs], [1, P]]),
                            op=AluOpType.is_equal)
                        gbase = sdone * tw
                        zps = ps.tile([P, 2048], F32, tag="zps", bufs=1)
                        for s in range(gs):
                            nc.tensor.matmul(
                                out=zps[:, 512 * s:512 * s + w],
                                lhsT=s01de[:, s * P:(s + 1) * P], rhs=xr_t,
                                start=True, stop=False)
                        for s in range(gs):
                            nc.tensor.matmul(
                                out=zps[:, 512 * s:512 * s + w],
                                lhsT=ident[:],
                                rhs=g_t[:, gbase + s * tw:gbase + s * tw + w],
                                start=False, stop=True)
                        rz = gat.tile([P, _GSUB * D_HID], BF16, tag="rz")
                        nc.scalar.activation(
                            out=_ap(rz[:], [[oc, gs], [1, oc]]),
                            in_=_ap(zps[:], [[512, gs], [1, oc]]),
                            func=mybir.ActivationFunctionType.Relu)
                        tp = gat.tile([P, _GSUB * D_HID], BF16, tag="tp")
                        nc.vector.tensor_tensor(
                            out=_ap(tp[:], [[oc, gs], [1, oc]]),
                            in0=_ap(rz[:], [[oc, gs], [1, oc]]),
                            in1=_ap(att08[:], [[0, gs], [1, oc]]),
                            op=AluOpType.mult)
                        sc = gat.tile([P, _GSUB * HEADS], F32, tag="sc")
                        nc.vector.tensor_reduce(
                            out=sc[:, :gs * heads],
                            in_=_ap(tp[:], [[C, gs * heads], [1, C]]),
                            axis=AX, op=AluOpType.add)
                        sc2 = gat.tile([P, _GSUB * HEADS], F32, tag="sc2")
                        nc.vector.scalar_tensor_tensor(
                            out=sc2[:, :gs * heads],
                            in0=_ap_off(zps[:], oc, [[512, gs], [1, heads]]),
                            scalar=0.2, in1=sc[:, :gs * heads],
                            op0=AluOpType.mult, op1=AluOpType.add)
                        msg = gat.tile([P, _GSUB * WMAX], BF16, tag="msg")
                        nc.scalar.activation(
                            out=_ap_off(msg[:], oc, [[w, gs], [1, heads]]),
                            in_=sc2[:, :gs * heads],
                            func=mybir.ActivationFunctionType.Exp)
                        nc.vector.tensor_tensor(
                            out=_ap(msg[:], [[w, gs], [1, oc]]),
                            in0=_ap_off(g_t[:], gbase, [[tw, gs], [1, oc]]),
                            in1=_ap_off(msg[:], oc, [[w, gs], [1, heads], [0, C]]),
                            op=AluOpType.mult)
                        for s in range(gs):
                            nc.tensor.matmul(
                                out=acc[:, 0:w],
                                lhsT=s01e[:, s * P:(s + 1) * P],
                                rhs=msg[:, s * w:s * w + w],
                                start=(sdone == 0 and s == 0),
                                stop=(g == ngr - 1 and s == gs - 1))
                        sdone += gs
                        gi += 1
                    coff += nsub
                    # -- epilogue
                    dn = wk.tile([P, HEADS], F32, tag="dn")
                    nc.vector.tensor_scalar_max(out=dn[:, :heads],
                                                in0=acc[:, oc:oc + heads],
                                                scalar1=1e-30)
                    rdn = wk.tile([P, HEADS], F32, tag="rdn")
                    nc.vector.reciprocal(out=rdn[:, :heads], in_=dn[:, :heads])
                    u = wk.tile([P, D_HID], F32, tag="u")
                    nc.vector.tensor_tensor(
                        out=u[:, :oc], in0=acc[:, 0:oc],
                        in1=_ap(rdn[:, :heads], [[1, heads], [0, C]]),
                        op=AluOpType.mult)
                    v = wk.tile([P, D_HID], F32, tag="v")
                    nc.vector.tensor_tensor(out=v[:, :oc], in0=u[:, :oc],
                                            in1=gbias[:], op=AluOpType.add)
                    if name != "l4":
                        hn = wk.tile([P, D_HID], BF16, tag="hn")
                        nc.vector.scalar_tensor_tensor(
                            out=hn[:, :oc], in0=v[:, :oc], scalar=0.01,
                            in1=v[:, :oc], op0=AluOpType.mult, op1=AluOpType.max)
                        for j in range(oc // P):
                            tps = ps.tile([P, P], BF16, tag="tps", bufs=2)
                            nc.tensor.transpose(tps[:], hn[:, j * P:(j + 1) * P],
                                                ident[:])
                            nc.vector.tensor_copy(
                                out=hT_next[:, j * NPAD + t * P:
                                            j * NPAD + (t + 1) * P],
                                in_=tps[:])
                    else:
                        hn4 = wk.tile([P, C], F32, tag="hn4")
                        nc.vector.scalar_tensor_tensor(
                            out=hn4[:], in0=v[:, :oc], scalar=0.01,
                            in1=v[:, :oc], op0=AluOpType.mult, op1=AluOpType.max)
                        hrt = hres[:, t * 2 * C:t * 2 * C + C]
                        nc.vector.tensor_tensor(
                            out=hrt, in0=hn4[:], in1=xrm[:, t * C:(t + 1) * C],
                            op=AluOpType.add)
                        nc.vector.tensor_tensor(
                            out=hres[:, t * 2 * C + C:(t + 1) * 2 * C],
                            in0=hrt, in1=hrt, op=AluOpType.mult)

            def lhsT_x(kt, t, kk):
                return xT[0:kk, t * P:(t + 1) * P]

            def lhsT_of(hT):
                def f(kt, t, kk):
                    return hT[0:kk, kt * NPAD + t * P:kt * NPAD + (t + 1) * P]
                return f

            layer("l0", lhsT_x, hT_a, 0)
            layer("l1", lhsT_of(hT_a), hT_b, 1)
            layer("l2", lhsT_of(hT_b), hT_a, 2)
            layer("l3", lhsT_of(hT_a), hT_b, 3)
            layer("l4", lhsT_of(hT_b), None, 4)

            # ---- final: graph LayerNorm + softmax aggregation
            pst = ps.tile([B, P], F32, tag="acc", bufs=2)
            for t in range(NT):
                nc.tensor.matmul(out=pst[:], lhsT=grm[:, t * B:(t + 1) * B],
                                 rhs=hres[:, t * 2 * C:(t + 1) * 2 * C],
                                 start=(t == 0), stop=(t == NT - 1))
            st_sb = wk.tile([B, P], F32, tag="st_sb")
            nc.vector.tensor_copy(out=st_sb[:], in_=pst[:])
            nc.sync.dma_start(out=st_in[:], in_=st_sb[:])
            nc.gpsimd.collective_compute(
                "AllReduce", AluOpType.add, replica_groups=[list(range(NC))],
                ins=[st_in[:]], outs=[st_out[:]])
            st_r = wk.tile([B, P], F32, tag="st_r")
            nc.sync.dma_start(out=st_r[:], in_=st_out[:])

            ssum = wk.tile([B, 4], F32, tag="ssum")
            nc.vector.tensor_reduce(out=ssum[:, 0:1], in_=st_r[:, 0:C],
                                    axis=AX, op=AluOpType.add)
            nc.vector.tensor_reduce(out=ssum[:, 1:2], in_=st_r[:, C:2 * C],
                                    axis=AX, op=AluOpType.add)
            mean = wk.tile([B, 1], F32, tag="mean")
            nc.vector.tensor_scalar(out=mean[:], in0=ssum[:, 0:1],
                                    scalar1=rcnt[:], scalar2=None,
                                    op0=AluOpType.mult)
            var = wk.tile([B, 1], F32, tag="var")
            nc.vector.tensor_scalar(out=var[:], in0=ssum[:, 1:2],
                                    scalar1=rcnt[:], scalar2=None,
                                    op0=AluOpType.mult)
            m2 = wk.tile([B, 1], F32, tag="m2")
            nc.vector.tensor_tensor(out=m2[:], in0=mean[:], in1=mean[:],
                                    op=AluOpType.mult)
            nc.vector.tensor_tensor(out=var[:], in0=var[:], in1=m2[:],
                                    op=AluOpType.subtract)
            nc.vector.tensor_scalar_add(out=var[:], in0=var[:], scalar1=1e-5)
            rv = wk.tile([B, 1], F32, tag="rv")
            nc.vector.reciprocal(out=rv[:], in_=var[:])
            rsq = wk.tile([B, 1], F32, tag="rsq")
            nc.scalar.activation(out=rsq[:], in_=rv[:],
                                 func=mybir.ActivationFunctionType.Sqrt)
            svec = wk.tile([B, 2], F32, tag="svec")
            nc.vector.tensor_copy(out=svec[:, 0:1], in_=rsq[:])
            nc.vector.tensor_tensor(out=svec[:, 1:2], in0=mean[:], in1=rsq[:],
                                    op=AluOpType.mult)
            nc.vector.tensor_scalar_mul(out=svec[:, 1:2], in0=svec[:, 1:2],
                                        scalar1=-1.0)

            psw = ps.tile([B, P], F32, tag="acc", bufs=2)
            for t in range(NT):
                psb = ps.tile([P, 2], F32, tag="tps", bufs=2)
                nc.tensor.matmul(out=psb[:], lhsT=gcm[:, t * P:(t + 1) * P],
                                 rhs=svec[:], start=True, stop=True)
                sbv = wk.tile([P, 2], F32, tag="sbv")
                nc.vector.tensor_copy(out=sbv[:], in_=psb[:])
                h1 = wk.tile([P, C], F32, tag="h1")
                nc.scalar.activation(out=h1[:],
                                     in_=hres[:, t * 2 * C:t * 2 * C + C],
                                     func=mybir.ActivationFunctionType.Identity,
                                     bias=sbv[:, 1:2], scale=sbv[:, 0:1])
                hl = wk.tile([P, C], F32, tag="hl")
                nc.vector.tensor_tensor(out=hl[:], in0=h1[:], in1=lnw[:],
                                        op=AluOpType.mult)
                nc.vector.tensor_tensor(out=hl[:], in0=hl[:], in1=lnb[:],
                                        op=AluOpType.add)
                wst = wk.tile([P, 2 * C], F32, tag="wst")
                nc.scalar.activation(out=wst[:, 0:C], in_=hl[:],
                                     func=mybir.ActivationFunctionType.Exp,
                                     scale=tsc[:])
                nc.vector.tensor_tensor(out=wst[:, C:2 * C], in0=wst[:, 0:C],
                                        in1=hl[:], op=AluOpType.mult)
                nc.tensor.matmul(out=psw[:], lhsT=grm[:, t * B:(t + 1) * B],
                                 rhs=wst[:], start=(t == 0), stop=(t == NT - 1))
            ws_sb = wk.tile([B, P], F32, tag="ws_sb")
            nc.vector.tensor_copy(out=ws_sb[:], in_=psw[:])
            nc.sync.dma_start(out=ws_in[:], in_=ws_sb[:])
            nc.gpsimd.collective_compute(
                "AllReduce", AluOpType.add, replica_groups=[list(range(NC))],
                ins=[ws_in[:]], outs=[ws_out[:]])
            ws_r = wk.tile([B, P], F32, tag="ws_r")
            nc.sync.dma_start(out=ws_r[:], in_=ws_out[:])
            dnf = wk.tile([B, C], F32, tag="dnf")
            nc.vector.tensor_scalar_add(out=dnf[:], in0=ws_r[:, 0:C],
                                        scalar1=1e-16)
            rrf = wk.tile([B, C], F32, tag="rrf")
            nc.vector.reciprocal(out=rrf[:], in_=dnf[:])
            outv = wk.tile([B, C], F32, tag="outv")
            nc.vector.tensor_tensor(out=outv[:], in0=ws_r[:, C:2 * C],
                                    in1=rrf[:], op=AluOpType.mult)
            nc.sync.dma_start(out=out_fin[:], in_=outv[:])

    nc.compile()
    return nc


# ---------------------------------------------------------------------------
# Entry point
# ---------------------------------------------------------------------------

def kernel(x, edge_index, batch, params):
    x = np.asarray(x, np.float32)
    batch_np = np.asarray(batch).astype(np.int64)
    srcw, dst_cols, dstrow, s_t, groups = _prep_edges(np.asarray(edge_index))
    lm = _prep_params(params)

    nc = _build_program(s_t, groups, lm)

    cnt = np.zeros(B); np.add.at(cnt, batch_np, 1.0)
    rcnt = (1.0 / np.maximum(cnt * C, 1.0)).astype(np.float32).reshape(B, 1)
    t_val = float(np.asarray(params["aggr_t"]))
    tsc = np.full((P, 1), t_val, np.float32)
    lnw = np.tile(np.asarray(params["ln_w"], np.float32).reshape(1, C), (P, 1))
    lnb = np.tile(np.asarray(params["ln_b"], np.float32).reshape(1, C), (P, 1))
    iota_row = np.tile(np.arange(P, dtype=np.float32)[None, :], (P, 1)).astype(NPBF)
    iota_col = np.arange(P, dtype=np.float32).reshape(P, 1).astype(NPBF)
    ident = np.eye(P, dtype=np.float32).astype(NPBF)

    in_maps = []
    for k in range(NC):
        lo, hi = k * NPC, (k + 1) * NPC
        xs = np.zeros((NPAD, C), np.float32)
        xs[:NPC] = x[lo:hi]
        gr = np.zeros((NPAD, B), np.float32)
        gr[np.arange(NPC), batch_np[lo:hi]] = 1.0
        im = {
            "xT_in": np.ascontiguousarray(xs.T).astype(NPBF),
            "xrm_in": xs,
            "srcw_in": srcw[k],
            "dstloc_in": dst_cols[k],
            "dstrow_in": dstrow[k],
            "iota_row_in": iota_row,
            "iota_col_in": iota_col,
            "ident_in": ident,
            "grm_in": gr,
            "gcm_in": np.ascontiguousarray(gr.T),
            "rcnt_in": rcnt,
            "tsc_in": tsc,
            "lnw_in": lnw,
            "lnb_in": lnb,
        }
        for name, m in lm.items():
            for wn in ["wl", "wr", "blb", "brb", "att08", "gbias"]:
                im[f"{name}_{wn}"] = np.ascontiguousarray(m[wn])
        in_maps.append(im)

    trace = os.environ.get("KERNEL_TRACE", "0") == "1"
    res = bass_utils.run_bass_kernel_spmd(
        nc, in_maps, core_ids=list(range(NC)), trace=trace)
    kernel.last_results = res
    return np.asarray(res.results[0]["out_fin"], np.float32)
